# revision 8
# baseline (speedup 1.0000x reference)
"""Llama GQA attention layer (S=2048, H=4096, 32 q heads / 8 kv heads, D=128)
on 8 Trainium2 NeuronCores.

Strategy:
  - Tensor-parallel by heads: core c owns q-heads 4c..4c+3 and kv-head c.
    Wqkv is column-sharded on the host into a per-core [4096, 768] slab
    (512 q cols | 128 k cols | 128 v cols), cast to bf16.
  - hidden_states is shipped pre-transposed ([H, S], bf16) so the QKV
    matmul needs no on-device transpose; RoPE is applied at PSUM-evict
    using host-built cos/sin tables ([128, S], f32).
  - Attention is computed per head in "scores-transposed" layout
    (k on partitions, q on free dim): sT = K^T.T @ Q^T, exp on ACT,
    causal mask via a sliding 0/1 mask multiply, PV and the softmax
    denominator both accumulate in PSUM via matmuls (ones-column trick),
    normalization fused into the PSUM evict.
  - The per-core attention outputs oT [512, 2048] are re-sharded from
    head-parallel to token-parallel with a single small AllToAll
    (bf16, 2.1 MB/core) instead of the 33 MB AllReduce a row-sharded
    o_proj would need.
  - Each core then computes its 256 output rows against the FULL Wo
    (bf16, streamed from HBM), and the host concatenates row shards.
"""
import sys

sys.path.insert(0, "/opt/trn_rl_repo")

from contextlib import ExitStack

import numpy as np

import concourse.bass as bass
import concourse.mybir as mybir
import concourse.tile as tile
from concourse import bacc
from concourse.bass_utils import run_bass_kernel_spmd
from concourse.masks import make_identity

BF16 = mybir.dt.bfloat16
F32 = mybir.dt.float32
NPBF16 = mybir.dt.np(BF16)

S = 2048          # sequence length
H = 4096          # hidden dim
D = 128           # head dim
NCORES = 8
HPC = 4           # q heads per core
QC = HPC * D      # 512 q cols per core
QKVC = QC + 2 * D  # 768 qkv cols per core
TB = 512          # token block (matmul free dim)
NTB = S // TB     # 4
NKT = H // 128    # 32 contraction tiles
TPC = S // NCORES  # 256 output tokens per core
SCALE = float(D) ** -0.5


def _build_nc(iters=1):
    nc = bacc.Bacc("TRN2", target_bir_lowering=False, debug=False,
                   num_devices=NCORES)

    hsT = nc.dram_tensor("hsT", [H, S], BF16, kind="ExternalInput").ap()
    wqkv = nc.dram_tensor("wqkv", [H, QKVC], BF16, kind="ExternalInput").ap()
    wo = nc.dram_tensor("wo", [H, H], BF16, kind="ExternalInput").ap()
    cos2 = nc.dram_tensor("cos2", [D, S], F32, kind="ExternalInput").ap()
    sin2 = nc.dram_tensor("sin2", [D, S], F32, kind="ExternalInput").ap()
    pmask = nc.dram_tensor("pmask", [128, 1280], BF16, kind="ExternalInput").ap()
    out = nc.dram_tensor("out", [TPC, H], F32, kind="ExternalOutput").ap()

    with tile.TileContext(nc) as tc:
        for _ in range(iters):
            with ExitStack() as ctx:
                _emit(ctx, tc, hsT, wqkv, wo, cos2, sin2, pmask, out)
    nc.compile()
    return nc


def _emit(ctx, tc, hsT, wqkv, wo, cos2, sin2, pmask, out):
    nc = tc.nc

    const = ctx.enter_context(tc.tile_pool(name="const", bufs=1))
    # whole Wqkv shard resident: [128, kt, 768] bf16 (6.3 MB)
    wq_sb = const.tile([128, NKT, QKVC], BF16)
    nc.sync.dma_start(out=wq_sb[:], in_=wqkv.rearrange("(kt p) c -> p kt c", p=128))
    cos_sb = const.tile([128, S], F32)
    nc.sync.dma_start(out=cos_sb[:], in_=cos2)
    sin_sb = const.tile([128, S], F32)
    nc.sync.dma_start(out=sin_sb[:], in_=sin2)
    mask_sb = const.tile([128, 1280], BF16)
    nc.sync.dma_start(out=mask_sb[:], in_=pmask)
    ones_sb = const.tile([128, 128], BF16)
    nc.gpsimd.memset(ones_sb[:], 1.0)
    ident_sb = const.tile([128, 128], BF16)
    make_identity(nc, ident_sb[:])

    # persistent activations
    acts = ctx.enter_context(tc.tile_pool(name="acts", bufs=1))
    qT = [acts.tile([128, S], BF16, name=f"qT{h}") for h in range(HPC)]
    kT = acts.tile([128, S], BF16)
    vS = acts.tile([128, 16 * 128], BF16)   # v token-major: [tok%128, (tokblk, d)]
    oT = [acts.tile([128, S], BF16, name=f"oT{h}") for h in range(HPC)]

    qkv_ctx = ExitStack()
    hs_pool = qkv_ctx.enter_context(tc.tile_pool(name="hs", bufs=2))
    qkv_psum = qkv_ctx.enter_context(tc.tile_pool(name="qkvps", bufs=3, space="PSUM"))
    ev_pool = qkv_ctx.enter_context(tc.tile_pool(name="ev", bufs=2))
    tp_psum = qkv_ctx.enter_context(tc.tile_pool(name="tpps", bufs=2, space="PSUM"))

    # ---- QKV projection + RoPE + V transpose ----
    for tb in range(NTB):
        hs_sb = hs_pool.tile([128, NKT, TB], BF16)
        nc.sync.dma_start(
            out=hs_sb[:],
            in_=hsT[:, tb * TB:(tb + 1) * TB].rearrange("(kt p) t -> p kt t", p=128),
        )
        for cb in range(6):
            ps = qkv_psum.tile([128, TB], F32)
            for kt in range(NKT):
                nc.tensor.matmul(
                    ps[:],
                    lhsT=wq_sb[:, kt, cb * 128:(cb + 1) * 128],
                    rhs=hs_sb[:, kt, :],
                    start=(kt == 0), stop=(kt == NKT - 1),
                )
            if cb < 5:
                # q head cb (cb<4) or k (cb==4): RoPE at evict
                s32 = ev_pool.tile([128, TB], F32, tag="s32")
                nc.scalar.copy(out=s32[:], in_=ps[:])
                qs = ev_pool.tile([128, TB], F32, tag="qs")
                nc.sync.dma_start(out=qs[0:64, :], in_=s32[64:128, :])
                nc.sync.dma_start(out=qs[64:128, :], in_=s32[0:64, :])
                t1 = ev_pool.tile([128, TB], F32, tag="t1")
                csl = slice(tb * TB, (tb + 1) * TB)
                nc.vector.tensor_mul(out=t1[:], in0=s32[:], in1=cos_sb[:, csl])
                t2 = ev_pool.tile([128, TB], F32, tag="t2")
                nc.vector.tensor_mul(out=t2[:], in0=qs[:], in1=sin_sb[:, csl])
                dst = qT[cb] if cb < HPC else kT
                nc.vector.tensor_sub(out=dst[:, csl], in0=t1[:], in1=t2[:])
            else:
                # v: evict bf16 then transpose [128,128] chunks to token-major
                vT = ev_pool.tile([128, TB], BF16, tag="vT")
                nc.scalar.copy(out=vT[:], in_=ps[:])
                for i in range(TB // 128):
                    tp = tp_psum.tile([128, 128], BF16)
                    nc.tensor.transpose(tp[:], vT[:, i * 128:(i + 1) * 128],
                                        ident_sb[:])
                    st = tb * 4 + i
                    nc.scalar.copy(out=vS[:, st * 128:(st + 1) * 128], in_=tp[:])

    qkv_ctx.close()

    # ---- attention (per head, scores-transposed flash style) ----
    at_ctx = ExitStack()
    at_psum = at_ctx.enter_context(tc.tile_pool(name="atps", bufs=3, space="PSUM"))
    acc_psum = at_ctx.enter_context(tc.tile_pool(name="accps", bufs=2, space="PSUM"))
    pr_pool = at_ctx.enter_context(tc.tile_pool(name="pr", bufs=3))
    nrm_pool = at_ctx.enter_context(tc.tile_pool(name="nrm", bufs=2))

    for h in range(HPC):
        for qt in range(NTB):
            o_ps = acc_psum.tile([128, TB], F32, tag="o")
            den_ps = acc_psum.tile([128, TB], F32, tag="den")
            nkt2 = 4 * qt + 4
            for kt2 in range(nkt2):
                s_ps = at_psum.tile([128, TB], F32)
                nc.tensor.matmul(
                    s_ps[:],
                    lhsT=kT[:, kt2 * 128:(kt2 + 1) * 128],
                    rhs=qT[h][:, qt * TB:(qt + 1) * TB],
                    start=True, stop=True,
                )
                pr = pr_pool.tile([128, TB], BF16)
                nc.scalar.activation(pr[:], s_ps[:],
                                     mybir.ActivationFunctionType.Exp, scale=SCALE)
                o = qt * TB - kt2 * 128
                if o <= 384:  # diagonal tile: apply causal mask
                    nc.vector.tensor_mul(
                        out=pr[:], in0=pr[:],
                        in1=mask_sb[:, 384 + o:384 + o + TB],
                    )
                nc.tensor.matmul(
                    o_ps[:], lhsT=vS[:, kt2 * 128:(kt2 + 1) * 128], rhs=pr[:],
                    start=(kt2 == 0), stop=(kt2 == nkt2 - 1), skip_group_check=True,
                )
                nc.tensor.matmul(
                    den_ps[:], lhsT=ones_sb[:], rhs=pr[:],
                    start=(kt2 == 0), stop=(kt2 == nkt2 - 1), skip_group_check=True,
                )
            rd = nrm_pool.tile([128, TB], F32)
            nc.vector.reciprocal(out=rd[:], in_=den_ps[:])
            nc.vector.tensor_mul(out=oT[h][:, qt * TB:(qt + 1) * TB],
                                 in0=o_ps[:], in1=rd[:])

    at_ctx.close()

    # ---- AllToAll: head-parallel -> token-parallel ----
    dram = ctx.enter_context(tc.tile_pool(name="dram", bufs=1, space="DRAM"))
    a2a_in = dram.tile([NCORES, QC, TPC], BF16)
    a2a_out = dram.tile([NCORES, QC, TPC], BF16)
    for h in range(HPC):
        for j in range(NCORES):
            nc.sync.dma_start(
                out=a2a_in[j, h * 128:(h + 1) * 128, :],
                in_=oT[h][:, j * TPC:(j + 1) * TPC],
            )
    nc.gpsimd.collective_compute(
        "AllToAll", mybir.AluOpType.bypass,
        replica_groups=[list(range(NCORES))],
        ins=[a2a_in.opt()], outs=[a2a_out.opt()],
    )

    # ---- output projection: out[tok 256, H] = oT_all.T @ Wo (full Wo) ----
    oL = acts.tile([128, NKT, TPC], BF16)
    nc.sync.dma_start(
        out=oL[:],
        in_=a2a_out.rearrange("g (k4 p) t -> p (g k4) t", p=128),
    )
    wo_pool = ctx.enter_context(tc.tile_pool(name="wo", bufs=6))
    out_psum = ctx.enter_context(tc.tile_pool(name="ops", bufs=2, space="PSUM"))
    res_pool = ctx.enter_context(tc.tile_pool(name="res", bufs=3))
    for ncb in range(H // TB):
        pss = [out_psum.tile([128, TB], F32, tag=f"po{th}", name=f"po{th}")
               for th in range(2)]
        for kt in range(NKT):
            wt = wo_pool.tile([128, TB], BF16)
            nc.sync.dma_start(
                out=wt[:],
                in_=wo[kt * 128:(kt + 1) * 128, ncb * TB:(ncb + 1) * TB],
            )
            for th in range(2):
                nc.tensor.matmul(
                    pss[th][:],
                    lhsT=oL[:, kt, th * 128:(th + 1) * 128], rhs=wt[:],
                    start=(kt == 0), stop=(kt == NKT - 1), skip_group_check=True,
                )
        for th in range(2):
            rs = res_pool.tile([128, TB], F32)
            nc.scalar.copy(out=rs[:], in_=pss[th][:])
            nc.sync.dma_start(
                out=out[th * 128:(th + 1) * 128, ncb * TB:(ncb + 1) * TB],
                in_=rs[:],
            )


_NC_CACHE = {}


def _get_nc():
    if "nc" not in _NC_CACHE:
        _NC_CACHE["nc"] = _build_nc()
    return _NC_CACHE["nc"]


def _host_prep(positions, hidden_states, Wqkv, Wo):
    positions = np.asarray(positions)
    hidden_states = np.asarray(hidden_states, dtype=np.float32)
    Wqkv = np.asarray(Wqkv, dtype=np.float32)
    Wo = np.asarray(Wo, dtype=np.float32)

    hsT = np.ascontiguousarray(hidden_states.T).astype(NPBF16)
    wo_bf = Wo.astype(NPBF16)

    q_size = 32 * D
    wqkv_shards = []
    for c in range(NCORES):
        qcols = Wqkv[:, c * QC:(c + 1) * QC]
        kcols = Wqkv[:, q_size + c * D:q_size + (c + 1) * D]
        vcols = Wqkv[:, q_size + 8 * D + c * D:q_size + 8 * D + (c + 1) * D]
        wqkv_shards.append(
            np.ascontiguousarray(np.concatenate([qcols, kcols, vcols], axis=1))
            .astype(NPBF16)
        )

    half = D // 2
    inv_freq = (1.0 / (10000.0 ** (np.arange(0, half, dtype=np.float32) / half))
                ).astype(np.float32)
    ang = positions.astype(np.float32)[:, None] * inv_freq[None, :]  # [S, 64]
    cosT = np.cos(ang).astype(np.float32).T  # [64, S]
    sinT = np.sin(ang).astype(np.float32).T
    cos2 = np.ascontiguousarray(np.vstack([cosT, cosT]))
    sin2 = np.ascontiguousarray(np.vstack([sinT, -sinT]))

    pm = (np.arange(128)[:, None] <= (np.arange(1280)[None, :] - 384))
    pmask = pm.astype(NPBF16)

    common = {"hsT": hsT, "wo": wo_bf, "cos2": cos2, "sin2": sin2, "pmask": pmask}
    return [dict(common, wqkv=wqkv_shards[c]) for c in range(NCORES)]


def kernel(positions, hidden_states, Wqkv, Wo):
    in_maps = _host_prep(positions, hidden_states, Wqkv, Wo)
    nc = _get_nc()
    res = run_bass_kernel_spmd(nc, in_maps, list(range(NCORES)))
    return np.concatenate([res.results[c]["out"] for c in range(NCORES)], axis=0)


# revision 12
# speedup vs baseline: 3.5504x; 3.5504x over previous
"""Llama GQA attention layer (S=2048, H=4096, 32 q heads / 8 kv heads, D=128)
on 8 Trainium2 NeuronCores.

Strategy:
  - Tensor-parallel by heads: core c owns q-heads 4c..4c+3 and kv-head c.
    Wqkv is column-sharded on the host into a per-core [4096, 768] slab
    (512 q cols | 128 k cols | 128 v cols), cast to bf16.
  - hidden_states is shipped pre-transposed ([H, S], bf16) so the QKV
    matmul needs no on-device transpose; RoPE is applied at PSUM-evict
    using host-built cos/sin tables ([128, S], f32).
  - Attention is computed per head in "scores-transposed" layout
    (k on partitions, q on free dim): sT = K^T.T @ Q^T, exp on ACT,
    causal mask via a sliding 0/1 mask multiply, PV and the softmax
    denominator both accumulate in PSUM via matmuls (ones-column trick),
    normalization fused into the PSUM evict.
  - The per-core attention outputs oT [512, 2048] are re-sharded from
    head-parallel to token-parallel with a single small AllToAll
    (bf16, 2.1 MB/core) instead of the 33 MB AllReduce a row-sharded
    o_proj would need.
  - Each core then computes its 256 output rows against the FULL Wo
    (bf16, streamed from HBM), and the host concatenates row shards.
"""
import sys

sys.path.insert(0, "/opt/trn_rl_repo")

from contextlib import ExitStack

import numpy as np

import concourse.bass as bass
import concourse.mybir as mybir
import concourse.tile as tile
from concourse import bacc
from concourse.bass_utils import run_bass_kernel_spmd
from concourse.masks import make_identity

BF16 = mybir.dt.bfloat16
F32 = mybir.dt.float32
NPBF16 = mybir.dt.np(BF16)

S = 2048          # sequence length
H = 4096          # hidden dim
D = 128           # head dim
NCORES = 8
HPC = 4           # q heads per core
QC = HPC * D      # 512 q cols per core
QKVC = QC + 2 * D  # 768 qkv cols per core
TB = 512          # token block (matmul free dim)
NTB = S // TB     # 4
NKT = H // 128    # 32 contraction tiles
TPC = S // NCORES  # 256 output tokens per core
SCALE = float(D) ** -0.5


def _build_nc(iters=1, nphases=4):
    nc = bacc.Bacc("TRN2", target_bir_lowering=False, debug=False,
                   num_devices=NCORES)

    hsT = nc.dram_tensor("hsT", [H, S], BF16, kind="ExternalInput").ap()
    wqkv = nc.dram_tensor("wqkv", [H, QKVC], BF16, kind="ExternalInput").ap()
    wo = nc.dram_tensor("wo", [H, H], BF16, kind="ExternalInput").ap()
    cos2 = nc.dram_tensor("cos2", [D, S], F32, kind="ExternalInput").ap()
    sin2 = nc.dram_tensor("sin2", [D, S], F32, kind="ExternalInput").ap()
    pmask = nc.dram_tensor("pmask", [128, 1280], BF16, kind="ExternalInput").ap()
    out = nc.dram_tensor("out", [TPC, H], F32, kind="ExternalOutput").ap()

    with tile.TileContext(nc) as tc:
        for _ in range(iters):
            with ExitStack() as ctx:
                _emit(ctx, tc, hsT, wqkv, wo, cos2, sin2, pmask, out, nphases)
    nc.compile()
    return nc


def _emit(ctx, tc, hsT, wqkv, wo, cos2, sin2, pmask, out, nphases=4):
    nc = tc.nc

    const = ctx.enter_context(tc.tile_pool(name="const", bufs=1))
    # whole Wqkv shard resident: [128, kt, 768] bf16 (6.3 MB)
    wq_sb = const.tile([128, NKT, QKVC], BF16)
    nc.sync.dma_start(out=wq_sb[:], in_=wqkv.rearrange("(kt p) c -> p kt c", p=128))
    cos_sb = const.tile([128, S], F32)
    nc.sync.dma_start(out=cos_sb[:], in_=cos2)
    sin_sb = const.tile([128, S], F32)
    nc.sync.dma_start(out=sin_sb[:], in_=sin2)
    mask_sb = const.tile([128, 1280], BF16)
    nc.sync.dma_start(out=mask_sb[:], in_=pmask)
    ones_sb = const.tile([128, 128], BF16)
    nc.gpsimd.memset(ones_sb[:], 1.0)
    ident_sb = const.tile([128, 128], BF16)
    make_identity(nc, ident_sb[:])

    # persistent activations
    acts = ctx.enter_context(tc.tile_pool(name="acts", bufs=1))
    qT = [acts.tile([128, S], BF16, name=f"qT{h}") for h in range(HPC)]
    kT = acts.tile([128, S], BF16)
    vS = acts.tile([128, 16 * 128], BF16)   # v token-major: [tok%128, (tokblk, d)]
    oT = [acts.tile([128, S], BF16, name=f"oT{h}") for h in range(HPC)]

    qkv_ctx = ExitStack()
    hs_pool = qkv_ctx.enter_context(tc.tile_pool(name="hs", bufs=2))
    qkv_psum = qkv_ctx.enter_context(tc.tile_pool(name="qkvps", bufs=3, space="PSUM"))
    ev_pool = qkv_ctx.enter_context(tc.tile_pool(name="ev", bufs=2))
    tp_psum = qkv_ctx.enter_context(tc.tile_pool(name="tpps", bufs=2, space="PSUM"))

    # ---- QKV projection + RoPE + V transpose ----
    for tb in range(NTB):
        hs_sb = hs_pool.tile([128, NKT, TB], BF16)
        nc.sync.dma_start(
            out=hs_sb[:],
            in_=hsT[:, tb * TB:(tb + 1) * TB].rearrange("(kt p) t -> p kt t", p=128),
        )
        for cb in range(6):
            ps = qkv_psum.tile([128, TB], F32)
            for kt in range(NKT):
                nc.tensor.matmul(
                    ps[:],
                    lhsT=wq_sb[:, kt, cb * 128:(cb + 1) * 128],
                    rhs=hs_sb[:, kt, :],
                    start=(kt == 0), stop=(kt == NKT - 1),
                )
            if cb < 5:
                # q head cb (cb<4) or k (cb==4): RoPE at evict
                s32 = ev_pool.tile([128, TB], F32, tag="s32")
                nc.scalar.copy(out=s32[:], in_=ps[:])
                qs = ev_pool.tile([128, TB], F32, tag="qs")
                nc.sync.dma_start(out=qs[0:64, :], in_=s32[64:128, :])
                nc.sync.dma_start(out=qs[64:128, :], in_=s32[0:64, :])
                t1 = ev_pool.tile([128, TB], F32, tag="t1")
                csl = slice(tb * TB, (tb + 1) * TB)
                nc.vector.tensor_mul(out=t1[:], in0=s32[:], in1=cos_sb[:, csl])
                t2 = ev_pool.tile([128, TB], F32, tag="t2")
                nc.vector.tensor_mul(out=t2[:], in0=qs[:], in1=sin_sb[:, csl])
                dst = qT[cb] if cb < HPC else kT
                nc.vector.tensor_sub(out=dst[:, csl], in0=t1[:], in1=t2[:])
            else:
                # v: evict bf16 then transpose [128,128] chunks to token-major
                vT = ev_pool.tile([128, TB], BF16, tag="vT")
                nc.scalar.copy(out=vT[:], in_=ps[:])
                for i in range(TB // 128):
                    tp = tp_psum.tile([128, 128], BF16)
                    nc.tensor.transpose(tp[:], vT[:, i * 128:(i + 1) * 128],
                                        ident_sb[:])
                    st = tb * 4 + i
                    nc.scalar.copy(out=vS[:, st * 128:(st + 1) * 128], in_=tp[:])

    qkv_ctx.close()
    if nphases < 2:
        # timing bisection: dump a QKV product so nothing is dead-code'd
        st = ctx.enter_context(tc.tile_pool(name="stg", bufs=2))
        for h in range(2):
            sg = st.tile([128, TPC], F32, tag="sg")
            nc.scalar.copy(out=sg[:], in_=qT[h][:, :TPC])
            nc.sync.dma_start(out=out[h * 128:(h + 1) * 128, :TPC], in_=sg[:])
        return

    # ---- attention (per head, scores-transposed flash style) ----
    at_ctx = ExitStack()
    at_psum = at_ctx.enter_context(tc.tile_pool(name="atps", bufs=3, space="PSUM"))
    acc_psum = at_ctx.enter_context(tc.tile_pool(name="accps", bufs=2, space="PSUM"))
    pr_pool = at_ctx.enter_context(tc.tile_pool(name="pr", bufs=3))
    nrm_pool = at_ctx.enter_context(tc.tile_pool(name="nrm", bufs=2))

    for h in range(HPC):
        for qt in range(NTB):
            o_ps = acc_psum.tile([128, TB], F32, tag="o")
            den_ps = acc_psum.tile([128, TB], F32, tag="den")
            nkt2 = 4 * qt + 4
            for kt2 in range(nkt2):
                s_ps = at_psum.tile([128, TB], F32)
                nc.tensor.matmul(
                    s_ps[:],
                    lhsT=kT[:, kt2 * 128:(kt2 + 1) * 128],
                    rhs=qT[h][:, qt * TB:(qt + 1) * TB],
                    start=True, stop=True,
                )
                pr = pr_pool.tile([128, TB], BF16)
                nc.scalar.activation(pr[:], s_ps[:],
                                     mybir.ActivationFunctionType.Exp, scale=SCALE)
                o = qt * TB - kt2 * 128
                if o <= 384:  # diagonal tile: apply causal mask
                    nc.vector.tensor_mul(
                        out=pr[:], in0=pr[:],
                        in1=mask_sb[:, 384 + o:384 + o + TB],
                    )
                nc.tensor.matmul(
                    o_ps[:], lhsT=vS[:, kt2 * 128:(kt2 + 1) * 128], rhs=pr[:],
                    start=(kt2 == 0), stop=(kt2 == nkt2 - 1), skip_group_check=True,
                )
                nc.tensor.matmul(
                    den_ps[:], lhsT=ones_sb[:], rhs=pr[:],
                    start=(kt2 == 0), stop=(kt2 == nkt2 - 1), skip_group_check=True,
                )
            rd = nrm_pool.tile([128, TB], F32)
            nc.vector.reciprocal(out=rd[:], in_=den_ps[:])
            nc.vector.tensor_mul(out=oT[h][:, qt * TB:(qt + 1) * TB],
                                 in0=o_ps[:], in1=rd[:])

    at_ctx.close()
    if nphases < 3:
        st = ctx.enter_context(tc.tile_pool(name="stg", bufs=2))
        for h in range(2):
            sg = st.tile([128, TPC], F32, tag="sg")
            nc.scalar.copy(out=sg[:], in_=oT[h][:, :TPC])
            nc.sync.dma_start(out=out[h * 128:(h + 1) * 128, :TPC], in_=sg[:])
        return

    # ---- AllToAll: head-parallel -> token-parallel ----
    dram = ctx.enter_context(tc.tile_pool(name="dram", bufs=1, space="DRAM"))
    a2a_in = dram.tile([NCORES, QC, TPC], BF16)
    a2a_out = dram.tile([NCORES, QC, TPC], BF16)
    for h in range(HPC):
        for j in range(NCORES):
            nc.sync.dma_start(
                out=a2a_in[j, h * 128:(h + 1) * 128, :],
                in_=oT[h][:, j * TPC:(j + 1) * TPC],
            )
    nc.gpsimd.collective_compute(
        "AllToAll", mybir.AluOpType.bypass,
        replica_groups=[list(range(NCORES))],
        ins=[a2a_in.opt()], outs=[a2a_out.opt()],
    )

    # ---- output projection: out[tok 256, H] = oT_all.T @ Wo (full Wo) ----
    oL = acts.tile([128, NKT, TPC], BF16)
    nc.sync.dma_start(
        out=oL[:],
        in_=a2a_out.rearrange("g (k4 p) t -> p (g k4) t", p=128),
    )
    if nphases < 4:
        st = ctx.enter_context(tc.tile_pool(name="stg", bufs=2))
        for h in range(2):
            sg = st.tile([128, TPC], F32, tag="sg")
            nc.scalar.copy(out=sg[:], in_=oL[:, h, :])
            nc.sync.dma_start(out=out[h * 128:(h + 1) * 128, :TPC], in_=sg[:])
        return
    wo_pool = ctx.enter_context(tc.tile_pool(name="wo", bufs=6))
    out_psum = ctx.enter_context(tc.tile_pool(name="ops", bufs=2, space="PSUM"))
    res_pool = ctx.enter_context(tc.tile_pool(name="res", bufs=3))
    for ncb in range(H // TB):
        pss = [out_psum.tile([128, TB], F32, tag=f"po{th}", name=f"po{th}")
               for th in range(2)]
        for kt in range(NKT):
            wt = wo_pool.tile([128, TB], BF16)
            nc.sync.dma_start(
                out=wt[:],
                in_=wo[kt * 128:(kt + 1) * 128, ncb * TB:(ncb + 1) * TB],
            )
            for th in range(2):
                nc.tensor.matmul(
                    pss[th][:],
                    lhsT=oL[:, kt, th * 128:(th + 1) * 128], rhs=wt[:],
                    start=(kt == 0), stop=(kt == NKT - 1), skip_group_check=True,
                )
        for th in range(2):
            rs = res_pool.tile([128, TB], F32)
            nc.scalar.copy(out=rs[:], in_=pss[th][:])
            nc.sync.dma_start(
                out=out[th * 128:(th + 1) * 128, ncb * TB:(ncb + 1) * TB],
                in_=rs[:],
            )


_NC_CACHE = {}


def _get_nc():
    if "nc" not in _NC_CACHE:
        _NC_CACHE["nc"] = _build_nc()
    return _NC_CACHE["nc"]


def _host_prep(positions, hidden_states, Wqkv, Wo):
    positions = np.asarray(positions)
    hidden_states = np.asarray(hidden_states, dtype=np.float32)
    Wqkv = np.asarray(Wqkv, dtype=np.float32)
    Wo = np.asarray(Wo, dtype=np.float32)

    hsT = np.ascontiguousarray(hidden_states.T).astype(NPBF16)
    wo_bf = Wo.astype(NPBF16)

    q_size = 32 * D
    wqkv_shards = []
    for c in range(NCORES):
        qcols = Wqkv[:, c * QC:(c + 1) * QC]
        kcols = Wqkv[:, q_size + c * D:q_size + (c + 1) * D]
        vcols = Wqkv[:, q_size + 8 * D + c * D:q_size + 8 * D + (c + 1) * D]
        wqkv_shards.append(
            np.ascontiguousarray(np.concatenate([qcols, kcols, vcols], axis=1))
            .astype(NPBF16)
        )

    half = D // 2
    inv_freq = (1.0 / (10000.0 ** (np.arange(0, half, dtype=np.float32) / half))
                ).astype(np.float32)
    ang = positions.astype(np.float32)[:, None] * inv_freq[None, :]  # [S, 64]
    cosT = np.cos(ang).astype(np.float32).T  # [64, S]
    sinT = np.sin(ang).astype(np.float32).T
    cos2 = np.ascontiguousarray(np.vstack([cosT, cosT]))
    sin2 = np.ascontiguousarray(np.vstack([sinT, -sinT]))

    pm = (np.arange(128)[:, None] <= (np.arange(1280)[None, :] - 384))
    pmask = pm.astype(NPBF16)

    common = {"hsT": hsT, "wo": wo_bf, "cos2": cos2, "sin2": sin2, "pmask": pmask}
    return [dict(common, wqkv=wqkv_shards[c]) for c in range(NCORES)]


def kernel(positions, hidden_states, Wqkv, Wo):
    in_maps = _host_prep(positions, hidden_states, Wqkv, Wo)
    nc = _get_nc()
    res = run_bass_kernel_spmd(nc, in_maps, list(range(NCORES)))
    return np.concatenate([res.results[c]["out"] for c in range(NCORES)], axis=0)


# revision 17
# speedup vs baseline: 3.8589x; 1.0869x over previous
"""Llama GQA attention layer (S=2048, H=4096, 32 q heads / 8 kv heads, D=128)
on 8 Trainium2 NeuronCores.

Strategy:
  - Tensor-parallel by heads: core c owns q-heads 4c..4c+3 and kv-head c.
    Wqkv is column-sharded on the host into a per-core [4096, 768] slab
    (512 q cols | 128 k cols | 128 v cols), cast to bf16.
  - hidden_states is shipped pre-transposed ([H, S], bf16) so the QKV
    matmul needs no on-device transpose; RoPE is applied at PSUM-evict
    using host-built cos/sin tables ([128, S], f32).
  - Attention is computed per head in "scores-transposed" layout
    (k on partitions, q on free dim): sT = K^T.T @ Q^T, exp on ACT,
    causal mask via a sliding 0/1 mask multiply, PV and the softmax
    denominator both accumulate in PSUM via matmuls (ones-column trick),
    normalization fused into the PSUM evict.
  - The per-core attention outputs oT [512, 2048] are re-sharded from
    head-parallel to token-parallel with a single small AllToAll
    (bf16, 2.1 MB/core) instead of the 33 MB AllReduce a row-sharded
    o_proj would need.
  - Each core then computes its 256 output rows against the FULL Wo
    (bf16, streamed from HBM), and the host concatenates row shards.
"""
import sys

sys.path.insert(0, "/opt/trn_rl_repo")

from contextlib import ExitStack

import numpy as np

import concourse.bass as bass
import concourse.mybir as mybir
import concourse.tile as tile
from concourse import bacc
from concourse.bass_utils import run_bass_kernel_spmd
from concourse.masks import make_identity

BF16 = mybir.dt.bfloat16
F32 = mybir.dt.float32
NPBF16 = mybir.dt.np(BF16)

S = 2048          # sequence length
H = 4096          # hidden dim
D = 128           # head dim
NCORES = 8
HPC = 4           # q heads per core
QC = HPC * D      # 512 q cols per core
QKVC = QC + 2 * D  # 768 qkv cols per core
TB = 512          # token block (matmul free dim)
NTB = S // TB     # 4
NKT = H // 128    # 32 contraction tiles
TPC = S // NCORES  # 256 output tokens per core
SCALE = float(D) ** -0.5


def _build_nc(iters=1, nphases=4):
    nc = bacc.Bacc("TRN2", target_bir_lowering=False, debug=False,
                   num_devices=NCORES)

    hsT = nc.dram_tensor("hsT", [H, S], BF16, kind="ExternalInput").ap()
    wqkv = nc.dram_tensor("wqkv", [H, QKVC], BF16, kind="ExternalInput").ap()
    wo = nc.dram_tensor("wo", [H, H], BF16, kind="ExternalInput").ap()
    cos2 = nc.dram_tensor("cos2", [D, S], F32, kind="ExternalInput").ap()
    sin2 = nc.dram_tensor("sin2", [D, S], F32, kind="ExternalInput").ap()
    pmask = nc.dram_tensor("pmask", [128, 1280], BF16, kind="ExternalInput").ap()
    out = nc.dram_tensor("out", [TPC, H], F32, kind="ExternalOutput").ap()

    with tile.TileContext(nc) as tc:
        for _ in range(iters):
            with ExitStack() as ctx:
                _emit(ctx, tc, hsT, wqkv, wo, cos2, sin2, pmask, out, nphases)
    nc.compile()
    return nc


def _emit(ctx, tc, hsT, wqkv, wo, cos2, sin2, pmask, out, nphases=4):
    nc = tc.nc

    const = ctx.enter_context(tc.tile_pool(name="const", bufs=1))
    # whole Wqkv shard resident: [128, kt, 768] bf16 (6.3 MB)
    wq_sb = const.tile([128, NKT, QKVC], BF16)
    nc.sync.dma_start(out=wq_sb[:], in_=wqkv.rearrange("(kt p) c -> p kt c", p=128))
    cos_sb = const.tile([128, S], F32)
    nc.sync.dma_start(out=cos_sb[:], in_=cos2)
    sin_sb = const.tile([128, S], F32)
    nc.sync.dma_start(out=sin_sb[:], in_=sin2)
    mask_sb = const.tile([128, 1280], BF16)
    nc.sync.dma_start(out=mask_sb[:], in_=pmask)
    ones_sb = const.tile([128, 128], BF16)
    nc.gpsimd.memset(ones_sb[:], 1.0)
    ident_sb = const.tile([128, 128], BF16)
    make_identity(nc, ident_sb[:])

    # persistent activations
    acts = ctx.enter_context(tc.tile_pool(name="acts", bufs=1))
    qT = [acts.tile([128, S], BF16, name=f"qT{h}") for h in range(HPC)]
    kT = acts.tile([128, S], BF16)
    vS = acts.tile([128, 16 * 128], BF16)   # v token-major: [tok%128, (tokblk, d)]
    oT = [acts.tile([128, S], BF16, name=f"oT{h}") for h in range(HPC)]

    qkv_ctx = ExitStack()
    hs_pool = qkv_ctx.enter_context(tc.tile_pool(name="hs", bufs=2))
    qkv_psum = qkv_ctx.enter_context(tc.tile_pool(name="qkvps", bufs=3, space="PSUM"))
    ev_pool = qkv_ctx.enter_context(tc.tile_pool(name="ev", bufs=2))
    tp_psum = qkv_ctx.enter_context(tc.tile_pool(name="tpps", bufs=2, space="PSUM"))

    # ---- QKV projection + RoPE + V transpose ----
    for tb in range(NTB):
        hs_sb = hs_pool.tile([128, NKT, TB], BF16)
        nc.sync.dma_start(
            out=hs_sb[:],
            in_=hsT[:, tb * TB:(tb + 1) * TB].rearrange("(kt p) t -> p kt t", p=128),
        )
        for cb in range(6):
            ps = qkv_psum.tile([128, TB], F32)
            for kt in range(NKT):
                nc.tensor.matmul(
                    ps[:],
                    lhsT=wq_sb[:, kt, cb * 128:(cb + 1) * 128],
                    rhs=hs_sb[:, kt, :],
                    start=(kt == 0), stop=(kt == NKT - 1),
                )
            if cb < 5:
                # q head cb (cb<4) or k (cb==4): RoPE at evict
                s32 = ev_pool.tile([128, TB], F32, tag="s32")
                nc.scalar.copy(out=s32[:], in_=ps[:])
                qs = ev_pool.tile([128, TB], F32, tag="qs")
                nc.sync.dma_start(out=qs[0:64, :], in_=s32[64:128, :])
                nc.sync.dma_start(out=qs[64:128, :], in_=s32[0:64, :])
                t1 = ev_pool.tile([128, TB], F32, tag="t1")
                csl = slice(tb * TB, (tb + 1) * TB)
                nc.vector.tensor_mul(out=t1[:], in0=s32[:], in1=cos_sb[:, csl])
                t2 = ev_pool.tile([128, TB], F32, tag="t2")
                nc.vector.tensor_mul(out=t2[:], in0=qs[:], in1=sin_sb[:, csl])
                dst = qT[cb] if cb < HPC else kT
                nc.vector.tensor_sub(out=dst[:, csl], in0=t1[:], in1=t2[:])
            else:
                # v: evict bf16 then transpose [128,128] chunks to token-major
                vT = ev_pool.tile([128, TB], BF16, tag="vT")
                nc.scalar.copy(out=vT[:], in_=ps[:])
                for i in range(TB // 128):
                    tp = tp_psum.tile([128, 128], BF16)
                    nc.tensor.transpose(tp[:], vT[:, i * 128:(i + 1) * 128],
                                        ident_sb[:])
                    st = tb * 4 + i
                    nc.scalar.copy(out=vS[:, st * 128:(st + 1) * 128], in_=tp[:])

    qkv_ctx.close()
    if nphases < 2:
        # timing bisection: dump a QKV product so nothing is dead-code'd
        st = ctx.enter_context(tc.tile_pool(name="stg", bufs=2))
        for h in range(2):
            sg = st.tile([128, TPC], F32, tag="sg")
            nc.scalar.copy(out=sg[:], in_=qT[h][:, :TPC])
            nc.sync.dma_start(out=out[h * 128:(h + 1) * 128, :TPC], in_=sg[:])
        return

    # ---- attention (per head, scores-transposed flash style) ----
    dram = ctx.enter_context(tc.tile_pool(name="dram", bufs=1, space="DRAM"))
    a2a_in = dram.tile([NCORES, QC, TPC], BF16)
    a2a_out = dram.tile([NCORES, QC, TPC], BF16)

    at_ctx = ExitStack()
    at_psum = at_ctx.enter_context(tc.tile_pool(name="atps", bufs=3, space="PSUM"))
    acc_psum = at_ctx.enter_context(tc.tile_pool(name="accps", bufs=2, space="PSUM"))
    pr_pool = at_ctx.enter_context(tc.tile_pool(name="pr", bufs=3))
    nrm_pool = at_ctx.enter_context(tc.tile_pool(name="nrm", bufs=2))

    for h in range(HPC):
        for qt in range(NTB):
            o_ps = acc_psum.tile([128, TB], F32, tag="o")
            den_ps = acc_psum.tile([128, TB], F32, tag="den")
            nkt2 = 4 * qt + 4

            def emit_pv(pr, kt2):
                nc.tensor.matmul(
                    o_ps[:], lhsT=vS[:, kt2 * 128:(kt2 + 1) * 128], rhs=pr[:],
                    start=(kt2 == 0), stop=(kt2 == nkt2 - 1), skip_group_check=True,
                )
                nc.tensor.matmul(
                    den_ps[:], lhsT=ones_sb[:], rhs=pr[:],
                    start=(kt2 == 0), stop=(kt2 == nkt2 - 1), skip_group_check=True,
                )

            # 1-deep software pipeline: QK(k+1) issues on PE before PV/den(k),
            # so the ACT exp of tile k hides behind tensor work.
            pending = None
            for kt2 in range(nkt2):
                s_ps = at_psum.tile([128, TB], F32)
                nc.tensor.matmul(
                    s_ps[:],
                    lhsT=kT[:, kt2 * 128:(kt2 + 1) * 128],
                    rhs=qT[h][:, qt * TB:(qt + 1) * TB],
                    start=True, stop=True,
                )
                pr = pr_pool.tile([128, TB], BF16)
                nc.scalar.activation(pr[:], s_ps[:],
                                     mybir.ActivationFunctionType.Exp, scale=SCALE)
                o = qt * TB - kt2 * 128
                if o <= 384:  # diagonal tile: apply causal mask
                    nc.vector.tensor_mul(
                        out=pr[:], in0=pr[:],
                        in1=mask_sb[:, 384 + o:384 + o + TB],
                    )
                if pending is not None:
                    emit_pv(*pending)
                pending = (pr, kt2)
            emit_pv(*pending)
            rd = nrm_pool.tile([128, TB], F32)
            nc.vector.reciprocal(out=rd[:], in_=den_ps[:])
            nc.vector.tensor_mul(out=oT[h][:, qt * TB:(qt + 1) * TB],
                                 in0=o_ps[:], in1=rd[:])
        if nphases >= 3:
            # ship this head's A2A input slices as soon as the head is done
            for j in range(NCORES):
                nc.sync.dma_start(
                    out=a2a_in[j, h * 128:(h + 1) * 128, :],
                    in_=oT[h][:, j * TPC:(j + 1) * TPC],
                )

    at_ctx.close()
    if nphases < 3:
        st = ctx.enter_context(tc.tile_pool(name="stg", bufs=2))
        for h in range(2):
            sg = st.tile([128, TPC], F32, tag="sg")
            nc.scalar.copy(out=sg[:], in_=oT[h][:, :TPC])
            nc.sync.dma_start(out=out[h * 128:(h + 1) * 128, :TPC], in_=sg[:])
        return

    # ---- AllToAll: head-parallel -> token-parallel ----
    nc.gpsimd.collective_compute(
        "AllToAll", mybir.AluOpType.bypass,
        replica_groups=[list(range(NCORES))],
        ins=[a2a_in.opt()], outs=[a2a_out.opt()],
    )

    # ---- output projection: out[tok 256, H] = oT_all.T @ Wo (full Wo) ----
    oL = acts.tile([128, NKT, TPC], BF16)
    nc.sync.dma_start(
        out=oL[:],
        in_=a2a_out.rearrange("g (k4 p) t -> p (g k4) t", p=128),
    )
    if nphases < 4:
        st = ctx.enter_context(tc.tile_pool(name="stg", bufs=2))
        for h in range(2):
            sg = st.tile([128, TPC], F32, tag="sg")
            nc.scalar.copy(out=sg[:], in_=oL[:, h, :])
            nc.sync.dma_start(out=out[h * 128:(h + 1) * 128, :TPC], in_=sg[:])
        return
    wo_pool = ctx.enter_context(tc.tile_pool(name="wo", bufs=2))
    out_psum = ctx.enter_context(tc.tile_pool(name="ops", bufs=2, space="PSUM"))
    res_pool = ctx.enter_context(tc.tile_pool(name="res", bufs=3))
    for ncb in range(H // TB):
        # one 4MB DMA per output-column block: wo[:, ncb*512:+512] as [p, kt, n]
        wt = wo_pool.tile([128, NKT, TB], BF16)
        nc.sync.dma_start(
            out=wt[:],
            in_=wo[:, ncb * TB:(ncb + 1) * TB].rearrange("(kt p) n -> p kt n", p=128),
        )
        pss = [out_psum.tile([128, TB], F32, tag=f"po{th}", name=f"po{th}")
               for th in range(2)]
        for kt in range(NKT):
            for th in range(2):
                nc.tensor.matmul(
                    pss[th][:],
                    lhsT=oL[:, kt, th * 128:(th + 1) * 128], rhs=wt[:, kt, :],
                    start=(kt == 0), stop=(kt == NKT - 1), skip_group_check=True,
                )
        for th in range(2):
            rs = res_pool.tile([128, TB], F32)
            nc.scalar.copy(out=rs[:], in_=pss[th][:])
            nc.sync.dma_start(
                out=out[th * 128:(th + 1) * 128, ncb * TB:(ncb + 1) * TB],
                in_=rs[:],
            )


_NC_CACHE = {}


def _get_nc():
    if "nc" not in _NC_CACHE:
        _NC_CACHE["nc"] = _build_nc()
    return _NC_CACHE["nc"]


def _host_prep(positions, hidden_states, Wqkv, Wo):
    positions = np.asarray(positions)
    hidden_states = np.asarray(hidden_states, dtype=np.float32)
    Wqkv = np.asarray(Wqkv, dtype=np.float32)
    Wo = np.asarray(Wo, dtype=np.float32)

    hsT = np.ascontiguousarray(hidden_states.T).astype(NPBF16)
    wo_bf = Wo.astype(NPBF16)

    q_size = 32 * D
    wqkv_shards = []
    for c in range(NCORES):
        qcols = Wqkv[:, c * QC:(c + 1) * QC]
        kcols = Wqkv[:, q_size + c * D:q_size + (c + 1) * D]
        vcols = Wqkv[:, q_size + 8 * D + c * D:q_size + 8 * D + (c + 1) * D]
        wqkv_shards.append(
            np.ascontiguousarray(np.concatenate([qcols, kcols, vcols], axis=1))
            .astype(NPBF16)
        )

    half = D // 2
    inv_freq = (1.0 / (10000.0 ** (np.arange(0, half, dtype=np.float32) / half))
                ).astype(np.float32)
    ang = positions.astype(np.float32)[:, None] * inv_freq[None, :]  # [S, 64]
    cosT = np.cos(ang).astype(np.float32).T  # [64, S]
    sinT = np.sin(ang).astype(np.float32).T
    cos2 = np.ascontiguousarray(np.vstack([cosT, cosT]))
    sin2 = np.ascontiguousarray(np.vstack([sinT, -sinT]))

    pm = (np.arange(128)[:, None] <= (np.arange(1280)[None, :] - 384))
    pmask = pm.astype(NPBF16)

    common = {"hsT": hsT, "wo": wo_bf, "cos2": cos2, "sin2": sin2, "pmask": pmask}
    return [dict(common, wqkv=wqkv_shards[c]) for c in range(NCORES)]


def kernel(positions, hidden_states, Wqkv, Wo):
    in_maps = _host_prep(positions, hidden_states, Wqkv, Wo)
    nc = _get_nc()
    res = run_bass_kernel_spmd(nc, in_maps, list(range(NCORES)))
    return np.concatenate([res.results[c]["out"] for c in range(NCORES)], axis=0)
